# revision 16
# baseline (speedup 1.0000x reference)
"""Trainium2 kernel for nn_Conv_RBS_state_vector.

The reference applies G=156 sequential RBS-gate unitaries (each d x d,
d = C(2I, 2) = 496) to a batch of state vectors.  Every RBS gate on the
Hamming-weight-2 subspace is the second exterior power (compound matrix)
of a plain Givens rotation on n = 2I qubits, so the whole circuit is

    U = Lambda^2(R),   R = G_156 ... G_1  (32 x 32 Givens product)

which collapses the computation to a single [B, d] @ [d, d] matmul.
The tiny theta-dependent setup (R, then U via the compound-matrix
formula) runs on host; the O(B d^2) matmul runs on the NeuronCores,
data-parallel over the batch (batch shard per core, U replicated).

Device-side design (per core, B_shard = 256, dp = 512):
  - everything in bf16 (quantization adds ~1e-3 rel err vs the 2e-2
    gate; accumulation stays fp32 in PSUM).
  - host pre-swizzles x and W into the exact SBUF tile layouts so each
    DMA is a single fully-contiguous transfer (8KB/partition runs) at
    near line rate, instead of the 512B-descriptor sprays a strided
    rearrange view produces.
  - batch-stationary matmul: lhsT = xT chunk [128 d_in, 128 batch],
    rhs = U^T chunk [128 d_in, 512 d_out] -> out y [128 batch, 512]
    accumulated over 4 d_in chunks.  8 LDWEIGHTS+MATMUL pairs total
    (vs 16 for the d_out-stationary variant) and 2 PSUM banks.
  - minimal instruction count: 3 DMAs in/out of DRAM + 8 matmuls +
    2 PSUM->SBUF cast-copies.  The end-of-NEFF event-semaphore
    teardown scales with instruction count and was ~8us of the
    22.6us baseline.
"""

import numpy as np
import ml_dtypes

import concourse.bacc as bacc
import concourse.bass as bass
import concourse.mybir as mybir
import concourse.tile as tile
from concourse.bass_utils import run_bass_kernel_spmd

N_CORES = 8
D = 496          # C(32, 2)
DP = 512         # zero-padded to a multiple of 128
B = 2048
B_SHARD = B // N_CORES   # 256
NK = 4           # contraction chunks of KC=124 partitions (496 = 4*124)
KC = 124         # chunk size; <=123 keeps partitions 124-127 off the DMA
NH = B_SHARD // 128  # 2 batch halves

BF16 = ml_dtypes.bfloat16

WARM_MMS = 7  # PE p-state warmup matmuls (see _make_nc)

_NC_CACHE: dict = {}


def _compound2(R: np.ndarray) -> np.ndarray:
    """Second compound matrix of R over the basis of pairs (a<b) in
    lexicographic order: U[(ab),(a'b')] = R[a,a']R[b,b'] - R[a,b']R[b,a']."""
    n = R.shape[0]
    a_of, b_of = np.triu_indices(n, k=1)
    return (
        R[np.ix_(a_of, a_of)] * R[np.ix_(b_of, b_of)]
        - R[np.ix_(a_of, b_of)] * R[np.ix_(b_of, a_of)]
    )


def _build_U(theta, M0, M1, M2, gate_tuple_idx, gate_param_idx) -> np.ndarray:
    """Compose the full-circuit unitary U (float64) on host.

    Primary path: derive the qubit q of each gate tuple from M1's sparsity
    pattern, build R as a product of Givens rotations, and take the second
    compound.  If any structural assumption fails, fall back to literal
    dense composition of the per-gate matrices (associativity only)."""
    M0 = np.asarray(M0)
    M1 = np.asarray(M1)
    M2 = np.asarray(M2)
    theta64 = np.asarray(theta, dtype=np.float64)
    gt = np.asarray(gate_tuple_idx).astype(np.int64)
    gp = np.asarray(gate_param_idx).astype(np.int64)
    T, d, _ = M0.shape

    try:
        n = int(round((1 + np.sqrt(1 + 8 * d)) / 2))
        assert n * (n - 1) // 2 == d
        a_of, b_of = np.triu_indices(n, k=1)
        q_of_t = np.zeros(T, np.int64)
        for t in range(T):
            nz = np.argwhere(M1[t] > 0.5)
            assert len(nz) > 0
            i, j = nz[0]
            diff = {a_of[i], b_of[i]} ^ {a_of[j], b_of[j]}
            q = min(diff)
            assert diff == {q, q + 1}
            q_of_t[t] = q

        c = np.cos(theta64)
        s = np.sin(theta64)
        R = np.eye(n, dtype=np.float64)
        for t_idx, p_idx in zip(gt, gp):
            q = q_of_t[t_idx]
            cg, sg = c[p_idx], s[p_idx]
            rq = R[q, :].copy()
            rq1 = R[q + 1, :].copy()
            R[q, :] = cg * rq + sg * rq1
            R[q + 1, :] = -sg * rq + cg * rq1
        return _compound2(R)
    except AssertionError:
        U = np.eye(d, dtype=np.float64)
        for t_idx, p_idx in zip(gt, gp):
            M = (
                M0[t_idx].astype(np.float64) * np.cos(theta64[p_idx])
                + M1[t_idx].astype(np.float64) * np.sin(theta64[p_idx])
                + M2[t_idx].astype(np.float64)
            )
            U = M @ U
        return U


def _make_nc():
    """SPMD program: y[b, n] = sum_k x[b, k] * U^T[k, n], bf16 in/out.

    DRAM layouts (pre-swizzled on host, all fully contiguous):
      xw  [128, 2048]: cols 0:1024  = x chunks, col (ki*2+h)*128 + b
                        holds x[h*128+b, ki*128+p] at partition p;
                        cols 1024:2048 = U^T chunks ki=0,1, col
                        1024 + ki*512 + n holds U[n, ki*128+p].
      w23 [128, 1024]: U^T chunks ki=2,3 (same layout).
      y   [128, 1024]: col h*512 + n holds y[h*128+b, n] at partition b.

    One HWDGE queue (SP) carries both input DMAs in consumption order so
    the first matmul can start after the first 512KB; the output DMA goes
    on ACT's queue.  Instruction count is kept minimal because the
    end-of-NEFF event-semaphore teardown scales with it.
    """
    nc = bacc.Bacc(None, target_bir_lowering=False)
    f32 = mybir.dt.float32
    bf16 = mybir.dt.bfloat16
    xw_d = nc.dram_tensor("xw", [KC, 2048], bf16, kind="ExternalInput")
    w23_d = nc.dram_tensor("w23", [KC, 1024], bf16, kind="ExternalInput")
    y0_d = nc.dram_tensor("y0", [128, DP], bf16, kind="ExternalOutput")
    y1_d = nc.dram_tensor("y1", [128, DP], bf16, kind="ExternalOutput")

    with tile.TileContext(nc) as tc:
        with (
            tc.tile_pool(name="xwp", bufs=1) as xwp,
            tc.tile_pool(name="wp", bufs=1) as wp,
            tc.tile_pool(name="yp", bufs=1) as yp,
            tc.tile_pool(name="ps", bufs=1, space="PSUM") as ps,
        ):
            xwt = xwp.tile([KC, 2048], bf16, tag="xw")
            wt1 = wp.tile([KC, 1024], bf16, tag="w23")
            warm = wp.tile([128, DP], bf16, tag="warm")
            yt0 = yp.tile([128, DP], bf16, tag="y0")
            yt1 = yp.tile([128, DP], bf16, tag="y1")
            yts = [yt0, yt1]
            # PE p-state warmup: the PE clock ramps 0.65 -> 1.2 -> 2.4 GHz
            # and only reaches full speed after ~3us of continuous busy.
            # Dummy matmuls on a memset tile keep the PE busy during the
            # input-DMA window so the real matmuls run at 2.4 GHz.
            nc.gpsimd.memset(warm[:], 0.0)
            # both input DMAs on SP's queue in consumption order: two
            # concurrent HWDGE queues round-robin at packet granularity
            # and halve each other's rate, which delays the first-needed
            # bytes; a single queue streams at full rate.
            # xw = x + U^T chunks ki=0,1 on SP's queue; U^T chunks
            # ki=2,3 concurrently on ACT's queue.
            nc.sync.dma_start(xwt[:], xw_d[:, :])
            nc.scalar.dma_start(wt1[:], w23_d[:, :])
            wacc = ps.tile([128, DP], f32)
            for _ in range(WARM_MMS):
                nc.tensor.matmul(
                    wacc[:], warm[:, :128], warm[:], start=True, stop=True
                )
            # h-outer so acc0 completes early and its copy + output DMA
            # overlap h=1's matmuls; both casts on DVE (it is free at each
            # point of use), per-half output DMAs on separate HWDGE queues.
            for h in range(NH):
                acc = ps.tile([128, DP], f32, tag=f"acc{h}")
                for ki in range(NK):
                    lhsT = xwt[:, (ki * NH + h) * 128 : (ki * NH + h + 1) * 128]
                    if ki < 2:
                        rhs = xwt[:, 1024 + ki * DP : 1024 + (ki + 1) * DP]
                    else:
                        rhs = wt1[:, (ki - 2) * DP : (ki - 1) * DP]
                    nc.tensor.matmul(
                        acc[:], lhsT, rhs, start=(ki == 0), stop=(ki == NK - 1)
                    )
                if h == 0:
                    nc.vector.tensor_copy(yts[h][:], acc[:])
                    nc.scalar.dma_start(y0_d[:, :], yts[h][:])
                else:
                    # split the critical-path cast across DVE and ACT
                    nc.vector.tensor_copy(yts[h][:, 0:256], acc[:, 0:256])
                    nc.scalar.copy(yts[h][:, 256:DP], acc[:, 256:DP])
                    nc.sync.dma_start(y1_d[:, :], yts[h][:])
    nc.compile()
    return nc


def _get_nc():
    if "nc" not in _NC_CACHE:
        _NC_CACHE["nc"] = _make_nc()
    return _NC_CACHE["nc"]


def _prep_w(U: np.ndarray) -> np.ndarray:
    """U (d x d, float64) -> [KC, NK*DP] bf16, w[p, ki*DP + n] = U[n, ki*KC+p]."""
    Wt = np.zeros((D, DP), np.float32)
    Wt[:, :D] = U.T.astype(np.float32)
    w = Wt.reshape(NK, KC, DP).transpose(1, 0, 2).reshape(KC, NK * DP)
    return np.ascontiguousarray(w).astype(BF16)


def _prep_x(xc: np.ndarray) -> np.ndarray:
    """x shard [256, d] fp32 -> [128, NK*NH*128] bf16,
    x[p, (ki*NH+h)*128 + b] = xc[h*128+b, ki*128+p]."""
    t = np.asarray(xc, np.float32).reshape(NH, 128, NK, KC)  # [h, b, ki, p]
    x = t.transpose(3, 2, 0, 1).reshape(KC, NK * NH * 128)
    return np.ascontiguousarray(x).astype(BF16)


def _run_device(x: np.ndarray, U: np.ndarray, trace: bool = False):
    """x: [B, d] fp32, U: [d, d] float64. Returns ([B, d] fp32, results)."""
    Bfull, d = x.shape
    assert d == D and Bfull == B, (x.shape,)
    w = _prep_w(U)              # [128, 2048]
    w01 = w[:, : 2 * DP]
    w23 = np.ascontiguousarray(w[:, 2 * DP :])
    nc = _get_nc()
    in_maps = []
    for c in range(N_CORES):
        xs = _prep_x(x[c * B_SHARD : (c + 1) * B_SHARD])
        xw = np.concatenate([xs, w01], axis=1)  # [128, 2048]
        in_maps.append({"xw": xw, "w23": w23})
    res = run_bass_kernel_spmd(nc, in_maps, core_ids=list(range(N_CORES)), trace=trace)
    outs = []
    for r in res.results:
        for h in range(NH):
            ys = np.asarray(r[f"y{h}"])  # [128, DP] bf16, partition = batch
            outs.append(ys[:, :D].astype(np.float32))
    return np.concatenate(outs, axis=0), res


def kernel(input_state, theta, M0, M1, M2, gate_tuple_idx, gate_param_idx):
    x = np.ascontiguousarray(np.asarray(input_state, dtype=np.float32))
    U = _build_U(theta, M0, M1, M2, gate_tuple_idx, gate_param_idx)
    out, _ = _run_device(x, U, trace=False)
    return out.astype(np.float32)


# revision 18
# speedup vs baseline: 1.1975x; 1.1975x over previous
"""Trainium2 kernel for nn_Conv_RBS_state_vector.

The reference applies G=156 sequential RBS-gate unitaries (each d x d,
d = C(2I, 2) = 496) to a batch of state vectors.  Every RBS gate on the
Hamming-weight-2 subspace is the second exterior power (compound matrix)
of a plain Givens rotation on n = 2I qubits, so the whole circuit is

    U = Lambda^2(R),   R = G_156 ... G_1  (32 x 32 Givens product)

which collapses the computation to a single [B, d] @ [d, d] matmul.
The tiny theta-dependent setup (R, then U via the compound-matrix
formula) runs on host; the O(B d^2) matmul runs on the NeuronCores,
data-parallel over the batch (batch shard per core, U replicated).

Device-side design (per core, B_shard = 256, dp = 512):
  - everything in bf16 (quantization adds ~1e-3 rel err vs the 2e-2
    gate; accumulation stays fp32 in PSUM).
  - host pre-swizzles x and W into the exact SBUF tile layouts so each
    DMA is a single fully-contiguous transfer (8KB/partition runs) at
    near line rate, instead of the 512B-descriptor sprays a strided
    rearrange view produces.
  - batch-stationary matmul: lhsT = xT chunk [128 d_in, 128 batch],
    rhs = U^T chunk [128 d_in, 512 d_out] -> out y [128 batch, 512]
    accumulated over 4 d_in chunks.  8 LDWEIGHTS+MATMUL pairs total
    (vs 16 for the d_out-stationary variant) and 2 PSUM banks.
  - minimal instruction count: 3 DMAs in/out of DRAM + 8 matmuls +
    2 PSUM->SBUF cast-copies.  The end-of-NEFF event-semaphore
    teardown scales with instruction count and was ~8us of the
    22.6us baseline.
"""

import numpy as np
import ml_dtypes

import concourse.bacc as bacc
import concourse.bass as bass
import concourse.mybir as mybir
import concourse.tile as tile
from concourse.bass_utils import run_bass_kernel_spmd

N_CORES = 8
D = 496          # C(32, 2)
DP = 512         # zero-padded to a multiple of 128
B = 2048
B_SHARD = B // N_CORES   # 256
NK = DP // 128   # 4 contraction chunks
NH = B_SHARD // 128  # 2 batch halves

BF16 = ml_dtypes.bfloat16

WARM_MMS = 7  # PE p-state warmup matmuls (see _make_nc)

_NC_CACHE: dict = {}


def _compound2(R: np.ndarray) -> np.ndarray:
    """Second compound matrix of R over the basis of pairs (a<b) in
    lexicographic order: U[(ab),(a'b')] = R[a,a']R[b,b'] - R[a,b']R[b,a']."""
    n = R.shape[0]
    a_of, b_of = np.triu_indices(n, k=1)
    return (
        R[np.ix_(a_of, a_of)] * R[np.ix_(b_of, b_of)]
        - R[np.ix_(a_of, b_of)] * R[np.ix_(b_of, a_of)]
    )


def _build_U(theta, M0, M1, M2, gate_tuple_idx, gate_param_idx) -> np.ndarray:
    """Compose the full-circuit unitary U (float64) on host.

    Primary path: derive the qubit q of each gate tuple from M1's sparsity
    pattern, build R as a product of Givens rotations, and take the second
    compound.  If any structural assumption fails, fall back to literal
    dense composition of the per-gate matrices (associativity only)."""
    M0 = np.asarray(M0)
    M1 = np.asarray(M1)
    M2 = np.asarray(M2)
    theta64 = np.asarray(theta, dtype=np.float64)
    gt = np.asarray(gate_tuple_idx).astype(np.int64)
    gp = np.asarray(gate_param_idx).astype(np.int64)
    T, d, _ = M0.shape

    try:
        n = int(round((1 + np.sqrt(1 + 8 * d)) / 2))
        assert n * (n - 1) // 2 == d
        a_of, b_of = np.triu_indices(n, k=1)
        q_of_t = np.zeros(T, np.int64)
        for t in range(T):
            nz = np.argwhere(M1[t] > 0.5)
            assert len(nz) > 0
            i, j = nz[0]
            diff = {a_of[i], b_of[i]} ^ {a_of[j], b_of[j]}
            q = min(diff)
            assert diff == {q, q + 1}
            q_of_t[t] = q

        c = np.cos(theta64)
        s = np.sin(theta64)
        R = np.eye(n, dtype=np.float64)
        for t_idx, p_idx in zip(gt, gp):
            q = q_of_t[t_idx]
            cg, sg = c[p_idx], s[p_idx]
            rq = R[q, :].copy()
            rq1 = R[q + 1, :].copy()
            R[q, :] = cg * rq + sg * rq1
            R[q + 1, :] = -sg * rq + cg * rq1
        return _compound2(R)
    except AssertionError:
        U = np.eye(d, dtype=np.float64)
        for t_idx, p_idx in zip(gt, gp):
            M = (
                M0[t_idx].astype(np.float64) * np.cos(theta64[p_idx])
                + M1[t_idx].astype(np.float64) * np.sin(theta64[p_idx])
                + M2[t_idx].astype(np.float64)
            )
            U = M @ U
        return U


def _make_nc():
    """SPMD program: y[b, n] = sum_k x[b, k] * U^T[k, n], bf16 in/out.

    DRAM layouts (pre-swizzled on host, all fully contiguous):
      xw  [128, 2048]: cols 0:1024  = x chunks, col (ki*2+h)*128 + b
                        holds x[h*128+b, ki*128+p] at partition p;
                        cols 1024:2048 = U^T chunks ki=0,1, col
                        1024 + ki*512 + n holds U[n, ki*128+p].
      w23 [128, 1024]: U^T chunks ki=2,3 (same layout).
      y   [128, 1024]: col h*512 + n holds y[h*128+b, n] at partition b.

    One HWDGE queue (SP) carries both input DMAs in consumption order so
    the first matmul can start after the first 512KB; the output DMA goes
    on ACT's queue.  Instruction count is kept minimal because the
    end-of-NEFF event-semaphore teardown scales with it.
    """
    nc = bacc.Bacc(None, target_bir_lowering=False)
    f32 = mybir.dt.float32
    bf16 = mybir.dt.bfloat16
    xw_d = nc.dram_tensor("xw", [128, 2048], bf16, kind="ExternalInput")
    w23_d = nc.dram_tensor("w23", [128, 1024], bf16, kind="ExternalInput")
    y0_d = nc.dram_tensor("y0", [128, DP], bf16, kind="ExternalOutput")
    y1_d = nc.dram_tensor("y1", [128, DP], bf16, kind="ExternalOutput")

    with tile.TileContext(nc) as tc:
        with (
            tc.tile_pool(name="xwp", bufs=1) as xwp,
            tc.tile_pool(name="wp", bufs=1) as wp,
            tc.tile_pool(name="yp", bufs=1) as yp,
            tc.tile_pool(name="ps", bufs=1, space="PSUM") as ps,
        ):
            xwt = xwp.tile([128, 2048], bf16, tag="xw")
            wt1 = wp.tile([128, 1024], bf16, tag="w23")
            warm = wp.tile([128, DP], bf16, tag="warm")
            yt0 = yp.tile([128, DP], bf16, tag="y0")
            yt1 = yp.tile([128, DP], bf16, tag="y1")
            yts = [yt0, yt1]
            # PE p-state warmup: the PE clock ramps 0.65 -> 1.2 -> 2.4 GHz
            # and only reaches full speed after ~3us of continuous busy.
            # Dummy matmuls on a memset tile keep the PE busy during the
            # input-DMA window so the real matmuls run at 2.4 GHz.
            nc.gpsimd.memset(warm[:], 0.0)
            # both input DMAs on SP's queue in consumption order: two
            # concurrent HWDGE queues round-robin at packet granularity
            # and halve each other's rate, which delays the first-needed
            # bytes; a single queue streams at full rate.
            # xw = x + U^T chunks ki=0,1 on SP's queue; U^T chunks
            # ki=2,3 concurrently on ACT's queue.
            nc.sync.dma_start(xwt[:], xw_d[:, :])
            nc.scalar.dma_start(wt1[:], w23_d[:, :])
            wacc = ps.tile([128, DP], f32)
            for _ in range(WARM_MMS):
                nc.tensor.matmul(
                    wacc[:], warm[:, :128], warm[:], start=True, stop=True
                )
            # h-outer so acc0 completes early and its copy + output DMA
            # overlap h=1's matmuls; both casts on DVE (it is free at each
            # point of use), per-half output DMAs on separate HWDGE queues.
            for h in range(NH):
                acc = ps.tile([128, DP], f32, tag=f"acc{h}")
                for ki in range(NK):
                    lhsT = xwt[:, (ki * NH + h) * 128 : (ki * NH + h + 1) * 128]
                    if ki < 2:
                        rhs = xwt[:, 1024 + ki * DP : 1024 + (ki + 1) * DP]
                    else:
                        rhs = wt1[:, (ki - 2) * DP : (ki - 1) * DP]
                    nc.tensor.matmul(
                        acc[:], lhsT, rhs, start=(ki == 0), stop=(ki == NK - 1)
                    )
                if h == 0:
                    nc.vector.tensor_copy(yts[h][:], acc[:])
                    nc.scalar.dma_start(y0_d[:, :], yts[h][:])
                else:
                    # the h=1 cast is on the critical path: split it across
                    # DVE and ACT so the two halves run concurrently
                    nc.vector.tensor_copy(yts[h][:, 0:256], acc[:, 0:256])
                    nc.scalar.copy(yts[h][:, 256:DP], acc[:, 256:DP])
                    nc.sync.dma_start(y1_d[:, :], yts[h][:])
    nc.compile()
    return nc


def _get_nc():
    if "nc" not in _NC_CACHE:
        _NC_CACHE["nc"] = _make_nc()
    return _NC_CACHE["nc"]


def _prep_w(U: np.ndarray) -> np.ndarray:
    """U (d x d, float64) -> [128, NK*DP] bf16, w[p, ki*DP + n] = U[n, ki*128+p]."""
    Wp = np.zeros((DP, DP), np.float32)
    Wp[:D, :D] = U.astype(np.float32)
    w = Wp.T.reshape(NK, 128, DP).transpose(1, 0, 2).reshape(128, NK * DP)
    return np.ascontiguousarray(w).astype(BF16)


def _prep_x(xc: np.ndarray) -> np.ndarray:
    """x shard [256, d] fp32 -> [128, NK*NH*128] bf16,
    x[p, (ki*NH+h)*128 + b] = xc[h*128+b, ki*128+p]."""
    xp_ = np.zeros((B_SHARD, DP), np.float32)
    xp_[:, :D] = xc
    t = xp_.reshape(NH, 128, NK, 128)  # [h, b, ki, p]
    x = t.transpose(3, 2, 0, 1).reshape(128, NK * NH * 128)
    return np.ascontiguousarray(x).astype(BF16)


def _run_device(x: np.ndarray, U: np.ndarray, trace: bool = False):
    """x: [B, d] fp32, U: [d, d] float64. Returns ([B, d] fp32, results)."""
    Bfull, d = x.shape
    assert d == D and Bfull == B, (x.shape,)
    w = _prep_w(U)              # [128, 2048]
    w01 = w[:, : 2 * DP]
    w23 = np.ascontiguousarray(w[:, 2 * DP :])
    nc = _get_nc()
    in_maps = []
    for c in range(N_CORES):
        xs = _prep_x(x[c * B_SHARD : (c + 1) * B_SHARD])
        xw = np.concatenate([xs, w01], axis=1)  # [128, 2048]
        in_maps.append({"xw": xw, "w23": w23})
    res = run_bass_kernel_spmd(nc, in_maps, core_ids=list(range(N_CORES)), trace=trace)
    outs = []
    for r in res.results:
        for h in range(NH):
            ys = np.asarray(r[f"y{h}"])  # [128, DP] bf16, partition = batch
            outs.append(ys[:, :D].astype(np.float32))
    return np.concatenate(outs, axis=0), res


def kernel(input_state, theta, M0, M1, M2, gate_tuple_idx, gate_param_idx):
    x = np.ascontiguousarray(np.asarray(input_state, dtype=np.float32))
    U = _build_U(theta, M0, M1, M2, gate_tuple_idx, gate_param_idx)
    out, _ = _run_device(x, U, trace=False)
    return out.astype(np.float32)
